# revision 3
# baseline (speedup 1.0000x reference)
"""KAN layer kernel for trn2 (8 NeuronCores, SPMD data-parallel over tokens).

Math: reference computes, per element x with t = tanh(x), u = 3.5*(t+1):
  out[n,o] = sum_i W[o,i] * (c0_i*B0(t_ni) + c1_i*B1(t_ni))
where B0/B1 are cubic B-splines on knots linspace(-1,1,8) (only first 2 of the
5 coeff columns are active: n_active = num_knots - spline_order = 2).

Closed form used here (cardinal cubic B-spline, symmetric form):
  B(w)   = (1/6) * (rho^3 - 4*sigma^3),  rho = relu(2-|w-2|), sigma = relu(1-|w-2|)
  B0(t)  = B(u)   -> |u-2| = |3.5t + 1.5|
  B1(t)  = B(u-1) -> |u-3| = |3.5t + 0.5|
The per-channel combine and the 1/6 fold into the output matmul:
  out = E0 @ M0T + E1 @ M1T,  E_k = rho_k^3 - 4 sigma_k^3,
  MkT[i,o] = W[o,i] * c_k[i] / 6.
E_k is computed with 2 custom DVE instructions per basis:
  KAN_R3 : rho^3   = v * relu(v)^2,         v = min(C1 - 3.5t, 3.5t + C2)
  KAN_S3A: Src1 + z^3,  z = min(max(C1' - m, m + C2'), 0) = cbrt(-4)*sigma
with cbrt(-4)^3 = -4 folding the -4 into the cube.
"""

import sys

sys.path.insert(0, "/opt/trn_rl_repo")

import numpy as np

CP = -(4.0 ** (1.0 / 3.0))  # cbrt(-4); z = CP*sigma -> z^3 = -4 sigma^3

N_CORES = 8
TOK_TOTAL = 16 * 4096
TOK_PER_CORE = TOK_TOTAL // N_CORES  # 8192
IN_DIM = 256
OUT_DIM = 256
CHUNK_TOK = 1024          # tokens per chunk
BLOCKS = CHUNK_TOK // 128  # 8 blocks per chunk
CHUNKS = TOK_PER_CORE // CHUNK_TOK  # 8 chunks per core

_CACHE = {}


def _register_ops():
    from concourse import dve_ops
    from concourse.dve_ops import DveOp, OPS, CUSTOM_DVE_SPECS
    from concourse.dve_spec import (
        Spec, Src0, Src1, C0, C1, C2, Zero, relu, sq, maxx, minn, lower,
        _has_src1,
    )
    from concourse.dve_uop import DveOpSpec

    def make(name, spec):
        if name in dve_ops._SUB_OPCODE_FOR_NAME:
            return next(op for op in OPS if op.name == name)
        row = dve_ops._CUSTOM_DVE_ROW_BASE + len(OPS)
        assert row < 0x20
        dve_ops._SUB_OPCODE_FOR_NAME[name] = row
        shas = {}
        for ver in ("v3", "v4"):
            tmp = DveOpSpec(
                name=name, opcode=row, uops=lower(spec, ver=ver),
                rd1_en=_has_src1(spec),
            )
            shas[ver] = tmp.sha(ver)
        op = DveOp(name, spec, subdim=False, uops_sha=shas)
        OPS.append(op)
        CUSTOM_DVE_SPECS[name] = spec
        return op

    # rho^3: m = 3.5t; v = min(C1 - m, m + C2); out = v*relu(v)^2
    m = Src0 * C0
    d = C1 - m
    g = m + C2
    v = minn(d, g)
    r3_body = v * sq(relu(v))

    def r3_ref(in0, in1, s0, s1, imm2):
        m = in0 * s0
        v = np.minimum(s1 - m, m + imm2)
        return (v * np.square(np.maximum(v, 0.0))).astype(np.float32)

    KAN_R3 = make("KAN_R3", Spec(body=r3_body, reference=r3_ref))

    # Src1 + z^3 with z = min(max(C1 - m, m + C2), 0), m = Src0*C0 (C0 = 3.5*CP)
    m2 = Src0 * C0
    d2 = C1 - m2
    g2 = m2 + C2
    z = minn(maxx(d2, g2), Zero)
    s3_body = Src1 + z * sq(z)

    def s3_ref(in0, in1, s0, s1, imm2):
        m = in0 * s0
        z = np.minimum(np.maximum(s1 - m, m + imm2), 0.0)
        return (in1 + z * np.square(z)).astype(np.float32)

    KAN_S3A = make("KAN_S3A", Spec(body=s3_body, reference=s3_ref))
    return KAN_R3, KAN_S3A


def _build_bass():
    import concourse.bass as bass
    import concourse.bacc as bacc
    import concourse.mybir as mybir
    from concourse import tile

    KAN_R3, KAN_S3A = _register_ops()

    f32 = mybir.dt.float32
    nc = bacc.Bacc(None, target_bir_lowering=False)

    xs = nc.dram_tensor("xs", [TOK_PER_CORE, IN_DIM], f32, kind="ExternalInput")
    m0t = nc.dram_tensor("m0t", [IN_DIM, OUT_DIM], f32, kind="ExternalInput")
    m1t = nc.dram_tensor("m1t", [IN_DIM, OUT_DIM], f32, kind="ExternalInput")
    ident = nc.dram_tensor("ident", [128, 128], f32, kind="ExternalInput")
    out = nc.dram_tensor("out", [TOK_PER_CORE, OUT_DIM], f32, kind="ExternalOutput")

    Tanh = mybir.ActivationFunctionType.Tanh

    with tile.TileContext(nc) as tc:
        with (
            tc.tile_pool(name="const", bufs=1) as cpool,
            tc.tile_pool(name="sbuf", bufs=2) as pool,
            tc.tile_pool(name="psum", bufs=2, space="PSUM") as ppool,
        ):
            idt = cpool.tile([128, 128], f32)
            nc.sync.dma_start(idt[:], ident[:])
            # weight halves: w[k][h] = MkT[h*128:(h+1)*128, :]
            wt = []
            for k, mt in enumerate((m0t, m1t)):
                row = []
                for h in range(2):
                    w = cpool.tile([128, OUT_DIM], f32, tag=f"w{k}{h}")
                    nc.sync.dma_start(w[:], mt[h * 128:(h + 1) * 128, :])
                    row.append(w)
                wt.append(row)

            for c in range(CHUNKS):
                xv = xs[c * CHUNK_TOK:(c + 1) * CHUNK_TOK, :].rearrange(
                    "(a p) i -> p a i", p=128
                )
                xt = pool.tile([128, BLOCKS * IN_DIM], f32, tag="xt")
                nc.sync.dma_start(xt[:].rearrange("p (a i) -> p a i", i=IN_DIM), xv)

                t = pool.tile([128, BLOCKS * IN_DIM], f32, tag="t")
                nc.scalar.activation(t[:], xt[:], Tanh)

                es = []
                for k, b in enumerate((1.5, 0.5)):
                    r3 = pool.tile([128, BLOCKS * IN_DIM], f32, tag=f"r3_{k}")
                    nc.vector._custom_dve(
                        KAN_R3, out=r3[:], in0=t[:],
                        s0=3.5, s1=2.0 - b, imm2=2.0 + b,
                    )
                    ek = pool.tile([128, BLOCKS * IN_DIM], f32, tag=f"e_{k}")
                    nc.vector._custom_dve(
                        KAN_S3A, out=ek[:], in0=t[:], in1=r3[:],
                        s0=3.5 * CP, s1=CP * (1.0 - b), imm2=CP * (1.0 + b),
                    )
                    es.append(ek)

                ov = out[c * CHUNK_TOK:(c + 1) * CHUNK_TOK, :].rearrange(
                    "(a p) o -> p a o", p=128
                )
                os_t = pool.tile([128, BLOCKS * OUT_DIM], f32, tag="os")
                for a in range(BLOCKS):
                    et = ppool.tile([128, 512], f32, tag="et")
                    for k in range(2):
                        for h in range(2):
                            nc.tensor.transpose(
                                et[:, (k * 2 + h) * 128:(k * 2 + h + 1) * 128],
                                es[k][:, a * IN_DIM + h * 128:a * IN_DIM + (h + 1) * 128],
                                idt[:],
                            )
                    ets = pool.tile([128, 512], f32, tag="ets")
                    nc.scalar.copy(ets[:], et[:])

                    acc = ppool.tile([128, OUT_DIM], f32, tag="acc")
                    for j in range(4):
                        k, h = j // 2, j % 2
                        nc.tensor.matmul(
                            acc[:],
                            ets[:, j * 128:(j + 1) * 128],
                            wt[k][h][:],
                            start=(j == 0),
                            stop=(j == 3),
                        )
                    nc.scalar.copy(os_t[:, a * OUT_DIM:(a + 1) * OUT_DIM], acc[:])
                nc.sync.dma_start(ov, os_t[:].rearrange("p (a o) -> p a o", o=OUT_DIM))

    nc.compile()
    return nc


def _get_nc():
    if "nc" not in _CACHE:
        _CACHE["nc"] = _build_bass()
    return _CACHE["nc"]


def kernel(x, inner_coeffs, outer_coeffs):
    from concourse import bass_utils

    x = np.asarray(x, dtype=np.float32)
    inner = np.asarray(inner_coeffs, dtype=np.float32)
    outer = np.asarray(outer_coeffs, dtype=np.float32)

    B, S, I = x.shape
    xf = np.ascontiguousarray(x.reshape(B * S, I))

    # MkT[i,o] = W[o,i] * c_k[i] / 6
    m0 = np.ascontiguousarray((outer.T * inner[:, 0:1]) / 6.0).astype(np.float32)
    m1 = np.ascontiguousarray((outer.T * inner[:, 1:2]) / 6.0).astype(np.float32)
    ident = np.eye(128, dtype=np.float32)

    nc = _get_nc()
    in_maps = []
    for i in range(N_CORES):
        in_maps.append({
            "xs": xf[i * TOK_PER_CORE:(i + 1) * TOK_PER_CORE],
            "m0t": m0, "m1t": m1, "ident": ident,
        })
    res = bass_utils.run_bass_kernel_spmd(nc, in_maps, list(range(N_CORES)))
    outs = [res.results[i]["out"] for i in range(N_CORES)]
    full = np.concatenate(outs, axis=0).reshape(B, S, OUT_DIM)
    return full


# revision 8
# speedup vs baseline: 1.2334x; 1.2334x over previous
"""KAN layer kernel for trn2 (8 NeuronCores, SPMD data-parallel over tokens).

Math: reference computes, per element x with t = tanh(x), u = 3.5*(t+1):
  out[n,o] = sum_i W[o,i] * (c0_i*B0(t_ni) + c1_i*B1(t_ni))
where B0/B1 are cubic B-splines on knots linspace(-1,1,8) (only first 2 of the
5 coeff columns are active: n_active = num_knots - spline_order = 2).

Closed form used here (cardinal cubic B-spline, symmetric form):
  B(w)   = (1/6) * (rho^3 - 4*sigma^3),  rho = relu(2-|w-2|), sigma = relu(1-|w-2|)
  B0(t)  = B(u)   -> |u-2| = |3.5t + 1.5|
  B1(t)  = B(u-1) -> |u-3| = |3.5t + 0.5|
The per-channel combine and the 1/6 fold into the output matmul:
  out = E0 @ M0T + E1 @ M1T,  E_k = rho_k^3 - 4 sigma_k^3,
  MkT[i,o] = W[o,i] * c_k[i] / 6.
E_k is computed with 2 custom DVE instructions per basis:
  KAN_R3 : rho^3   = v * relu(v)^2,         v = min(C1 - 3.5t, 3.5t + C2)
  KAN_S3A: Src1 + z^3,  z = min(max(C1' - m, m + C2'), 0) = cbrt(-4)*sigma
with cbrt(-4)^3 = -4 folding the -4 into the cube.
"""

import sys

sys.path.insert(0, "/opt/trn_rl_repo")

import numpy as np

CP = -(4.0 ** (1.0 / 3.0))  # cbrt(-4); z = CP*sigma -> z^3 = -4 sigma^3

N_CORES = 8
TOK_TOTAL = 16 * 4096
TOK_PER_CORE = TOK_TOTAL // N_CORES  # 8192
IN_DIM = 256
OUT_DIM = 256
CHUNK_TOK = 1024          # tokens per chunk
BLOCKS = CHUNK_TOK // 128  # 8 blocks per chunk
CHUNKS = TOK_PER_CORE // CHUNK_TOK  # 8 chunks per core

_CACHE = {}


def _register_ops():
    from concourse import dve_ops
    from concourse.dve_ops import DveOp, OPS, CUSTOM_DVE_SPECS
    from concourse.dve_spec import (
        Spec, Src0, Src1, C0, C1, C2, Zero, relu, sq, maxx, minn, lower,
        _has_src1,
    )
    from concourse.dve_uop import DveOpSpec

    def make(name, spec):
        if name in dve_ops._SUB_OPCODE_FOR_NAME:
            return next(op for op in OPS if op.name == name)
        row = dve_ops._CUSTOM_DVE_ROW_BASE + len(OPS)
        assert row < 0x20
        dve_ops._SUB_OPCODE_FOR_NAME[name] = row
        shas = {}
        for ver in ("v3", "v4"):
            tmp = DveOpSpec(
                name=name, opcode=row, uops=lower(spec, ver=ver),
                rd1_en=_has_src1(spec),
            )
            shas[ver] = tmp.sha(ver)
        op = DveOp(name, spec, subdim=False, uops_sha=shas)
        OPS.append(op)
        CUSTOM_DVE_SPECS[name] = spec
        return op

    # rho^3: m = 3.5t; v = min(C1 - m, m + C2); out = v*relu(v)^2
    m = Src0 * C0
    d = C1 - m
    g = m + C2
    v = minn(d, g)
    r3_body = v * sq(relu(v))

    def r3_ref(in0, in1, s0, s1, imm2):
        m = in0 * s0
        v = np.minimum(s1 - m, m + imm2)
        return (v * np.square(np.maximum(v, 0.0))).astype(np.float32)

    KAN_R3 = make("KAN_R3", Spec(body=r3_body, reference=r3_ref))

    # Src1 + z^3 with z = min(max(C1 - m, m + C2), 0), m = Src0*C0 (C0 = 3.5*CP)
    m2 = Src0 * C0
    d2 = C1 - m2
    g2 = m2 + C2
    z = minn(maxx(d2, g2), Zero)
    s3_body = Src1 + z * sq(z)

    def s3_ref(in0, in1, s0, s1, imm2):
        m = in0 * s0
        z = np.minimum(np.maximum(s1 - m, m + imm2), 0.0)
        return (in1 + z * np.square(z)).astype(np.float32)

    KAN_S3A = make("KAN_S3A", Spec(body=s3_body, reference=s3_ref))
    return KAN_R3, KAN_S3A


def _build_bass(sbufs=2, pbufs=2, chunk_tok=CHUNK_TOK):
    import concourse.bass as bass
    import concourse.bacc as bacc
    import concourse.mybir as mybir
    from concourse import tile

    blocks = chunk_tok // 128
    chunks = TOK_PER_CORE // chunk_tok

    KAN_R3, KAN_S3A = _register_ops()

    f32 = mybir.dt.float32
    nc = bacc.Bacc(None, target_bir_lowering=False)

    xs = nc.dram_tensor("xs", [TOK_PER_CORE, IN_DIM], f32, kind="ExternalInput")
    m0t = nc.dram_tensor("m0t", [IN_DIM, OUT_DIM], f32, kind="ExternalInput")
    m1t = nc.dram_tensor("m1t", [IN_DIM, OUT_DIM], f32, kind="ExternalInput")
    ident = nc.dram_tensor("ident", [128, 128], f32, kind="ExternalInput")
    out = nc.dram_tensor("out", [TOK_PER_CORE, OUT_DIM], f32, kind="ExternalOutput")

    Tanh = mybir.ActivationFunctionType.Tanh

    with tile.TileContext(nc) as tc:
        with (
            tc.tile_pool(name="const", bufs=1) as cpool,
            tc.tile_pool(name="sbuf", bufs=sbufs) as pool,
            tc.tile_pool(name="psum", bufs=pbufs, space="PSUM") as ppool,
        ):
            idt = cpool.tile([128, 128], f32)
            nc.sync.dma_start(idt[:], ident[:])
            # weight halves: w[k][h] = MkT[h*128:(h+1)*128, :]
            wt = []
            for k, mt in enumerate((m0t, m1t)):
                row = []
                for h in range(2):
                    w = cpool.tile([128, OUT_DIM], f32, tag=f"w{k}{h}")
                    nc.sync.dma_start(w[:], mt[h * 128:(h + 1) * 128, :])
                    row.append(w)
                wt.append(row)

            for c in range(chunks):
                xv = xs[c * chunk_tok:(c + 1) * chunk_tok, :].rearrange(
                    "(a p) i -> p a i", p=128
                )
                xt = pool.tile([128, blocks * IN_DIM], f32, tag="xt")
                nc.sync.dma_start(xt[:].rearrange("p (a i) -> p a i", i=IN_DIM), xv)

                # transpose x blocks: XT[h] = x[:, h-half].T, packed [128i, blocks*128tok]
                xtp0 = ppool.tile([128, blocks * 128], f32, tag="xtp0")
                xtp1 = ppool.tile([128, blocks * 128], f32, tag="xtp1")
                xtp = [xtp0, xtp1]
                for a in range(blocks):
                    for h in range(2):
                        nc.tensor.transpose(
                            xtp[h][:, a * 128:(a + 1) * 128],
                            xt[:, a * IN_DIM + h * 128:a * IN_DIM + (h + 1) * 128],
                            idt[:],
                        )
                # tanh fused with PSUM->SBUF: t2 cols = (h, tok)
                t2 = pool.tile([128, 2 * blocks * 128], f32, tag="t2")
                half = blocks * 128
                for h in range(2):
                    nc.scalar.activation(t2[:, h * half:(h + 1) * half],
                                         xtp[h][:], Tanh)

                es = []
                for k, b in enumerate((1.5, 0.5)):
                    r3 = pool.tile([128, 2 * half], f32, tag=f"r3_{k}")
                    nc.vector._custom_dve(
                        KAN_R3, out=r3[:], in0=t2[:],
                        s0=3.5, s1=2.0 - b, imm2=2.0 + b,
                    )
                    ek = pool.tile([128, 2 * half], f32, tag=f"e_{k}")
                    nc.vector._custom_dve(
                        KAN_S3A, out=ek[:], in0=t2[:], in1=r3[:],
                        s0=3.5 * CP, s1=CP * (1.0 - b), imm2=CP * (1.0 + b),
                    )
                    es.append(ek)

                ov = out[c * chunk_tok:(c + 1) * chunk_tok, :].rearrange(
                    "(a p) o -> p a o", p=128
                )
                os_t = pool.tile([128, blocks * OUT_DIM], f32, tag="os")
                for a in range(blocks):
                    acc = ppool.tile([128, OUT_DIM], f32, tag="acc")
                    for j in range(4):
                        k, h = j // 2, j % 2
                        nc.tensor.matmul(
                            acc[:],
                            es[k][:, h * half + a * 128:h * half + (a + 1) * 128],
                            wt[k][h][:],
                            start=(j == 0),
                            stop=(j == 3),
                        )
                    nc.scalar.copy(os_t[:, a * OUT_DIM:(a + 1) * OUT_DIM], acc[:])
                nc.sync.dma_start(ov, os_t[:].rearrange("p (a o) -> p a o", o=OUT_DIM))

    nc.compile()
    return nc


def _get_nc():
    if "nc" not in _CACHE:
        _CACHE["nc"] = _build_bass(SBUFS, PBUFS, CHUNK_TOK_RT)
    return _CACHE["nc"]


SBUFS = 4
PBUFS = 2
CHUNK_TOK_RT = 512


def kernel(x, inner_coeffs, outer_coeffs):
    from concourse import bass_utils

    x = np.asarray(x, dtype=np.float32)
    inner = np.asarray(inner_coeffs, dtype=np.float32)
    outer = np.asarray(outer_coeffs, dtype=np.float32)

    B, S, I = x.shape
    xf = np.ascontiguousarray(x.reshape(B * S, I))

    # MkT[i,o] = W[o,i] * c_k[i] / 6
    m0 = np.ascontiguousarray((outer.T * inner[:, 0:1]) / 6.0).astype(np.float32)
    m1 = np.ascontiguousarray((outer.T * inner[:, 1:2]) / 6.0).astype(np.float32)
    ident = np.eye(128, dtype=np.float32)

    nc = _get_nc()
    in_maps = []
    for i in range(N_CORES):
        in_maps.append({
            "xs": xf[i * TOK_PER_CORE:(i + 1) * TOK_PER_CORE],
            "m0t": m0, "m1t": m1, "ident": ident,
        })
    res = bass_utils.run_bass_kernel_spmd(nc, in_maps, list(range(N_CORES)))
    outs = [res.results[i]["out"] for i in range(N_CORES)]
    full = np.concatenate(outs, axis=0).reshape(B, S, OUT_DIM)
    return full


# revision 9
# speedup vs baseline: 26930.1662x; 21833.5716x over previous
"""KAN layer kernel for trn2 (8 NeuronCores, SPMD data-parallel over tokens).

Math: reference computes, per element x with t = tanh(x), u = 3.5*(t+1):
  out[n,o] = sum_i W[o,i] * (c0_i*B0(t_ni) + c1_i*B1(t_ni))
where B0/B1 are cubic B-splines on knots linspace(-1,1,8) (only first 2 of the
5 coeff columns are active: n_active = num_knots - spline_order = 2).

Closed form used here (cardinal cubic B-spline, symmetric form):
  B(w)   = (1/6) * (rho^3 - 4*sigma^3),  rho = relu(2-|w-2|), sigma = relu(1-|w-2|)
  B0(t)  = B(u)   -> |u-2| = |3.5t + 1.5|
  B1(t)  = B(u-1) -> |u-3| = |3.5t + 0.5|
The per-channel combine and the 1/6 fold into the output matmul:
  out = E0 @ M0T + E1 @ M1T,  E_k = rho_k^3 - 4 sigma_k^3,
  MkT[i,o] = W[o,i] * c_k[i] / 6.
E_k is computed with 2 custom DVE instructions per basis:
  KAN_R3 : rho^3   = v * relu(v)^2,         v = min(C1 - 3.5t, 3.5t + C2)
  KAN_S3A: Src1 + z^3,  z = min(max(C1' - m, m + C2'), 0) = cbrt(-4)*sigma
with cbrt(-4)^3 = -4 folding the -4 into the cube.
"""

import sys

sys.path.insert(0, "/opt/trn_rl_repo")

import numpy as np

CP = -(4.0 ** (1.0 / 3.0))  # cbrt(-4); z = CP*sigma -> z^3 = -4 sigma^3

N_CORES = 8
TOK_TOTAL = 16 * 4096
TOK_PER_CORE = TOK_TOTAL // N_CORES  # 8192
IN_DIM = 256
OUT_DIM = 256
CHUNK_TOK = 1024          # tokens per chunk
BLOCKS = CHUNK_TOK // 128  # 8 blocks per chunk
CHUNKS = TOK_PER_CORE // CHUNK_TOK  # 8 chunks per core

_CACHE = {}


def _register_ops():
    from concourse import dve_ops
    from concourse.dve_ops import DveOp, OPS, CUSTOM_DVE_SPECS
    from concourse.dve_spec import (
        Spec, Src0, Src1, C0, C1, C2, Zero, relu, sq, maxx, minn, lower,
        _has_src1,
    )
    from concourse.dve_uop import DveOpSpec

    def make(name, spec):
        if name in dve_ops._SUB_OPCODE_FOR_NAME:
            return next(op for op in OPS if op.name == name)
        row = dve_ops._CUSTOM_DVE_ROW_BASE + len(OPS)
        assert row < 0x20
        dve_ops._SUB_OPCODE_FOR_NAME[name] = row
        shas = {}
        for ver in ("v3", "v4"):
            tmp = DveOpSpec(
                name=name, opcode=row, uops=lower(spec, ver=ver),
                rd1_en=_has_src1(spec),
            )
            shas[ver] = tmp.sha(ver)
        op = DveOp(name, spec, subdim=False, uops_sha=shas)
        OPS.append(op)
        CUSTOM_DVE_SPECS[name] = spec
        return op

    # rho^3: m = 3.5t; v = min(C1 - m, m + C2); out = v*relu(v)^2
    m = Src0 * C0
    d = C1 - m
    g = m + C2
    v = minn(d, g)
    r3_body = v * sq(relu(v))

    def r3_ref(in0, in1, s0, s1, imm2):
        m = in0 * s0
        v = np.minimum(s1 - m, m + imm2)
        return (v * np.square(np.maximum(v, 0.0))).astype(np.float32)

    KAN_R3 = make("KAN_R3", Spec(body=r3_body, reference=r3_ref))

    # Src1 + z^3 with z = min(max(C1 - m, m + C2), 0), m = Src0*C0 (C0 = 3.5*CP)
    m2 = Src0 * C0
    d2 = C1 - m2
    g2 = m2 + C2
    z = minn(maxx(d2, g2), Zero)
    s3_body = Src1 + z * sq(z)

    def s3_ref(in0, in1, s0, s1, imm2):
        m = in0 * s0
        z = np.minimum(np.maximum(s1 - m, m + imm2), 0.0)
        return (in1 + z * np.square(z)).astype(np.float32)

    KAN_S3A = make("KAN_S3A", Spec(body=s3_body, reference=s3_ref))
    return KAN_R3, KAN_S3A


def _build_bass(sbufs=2, pbufs=2, chunk_tok=CHUNK_TOK):
    import concourse.bass as bass
    import concourse.bacc as bacc
    import concourse.mybir as mybir
    from concourse import tile

    blocks = chunk_tok // 128
    chunks = TOK_PER_CORE // chunk_tok

    KAN_R3, KAN_S3A = _register_ops()

    f32 = mybir.dt.float32
    nc = bacc.Bacc(None, target_bir_lowering=False)

    xs = nc.dram_tensor("xs", [TOK_PER_CORE, IN_DIM], f32, kind="ExternalInput")
    m0t = nc.dram_tensor("m0t", [IN_DIM, OUT_DIM], f32, kind="ExternalInput")
    m1t = nc.dram_tensor("m1t", [IN_DIM, OUT_DIM], f32, kind="ExternalInput")
    ident = nc.dram_tensor("ident", [128, 128], f32, kind="ExternalInput")
    out = nc.dram_tensor("out", [TOK_PER_CORE, OUT_DIM], f32, kind="ExternalOutput")

    Tanh = mybir.ActivationFunctionType.Tanh

    with tile.TileContext(nc) as tc:
        with (
            tc.tile_pool(name="const", bufs=1) as cpool,
            tc.tile_pool(name="sbuf", bufs=sbufs) as pool,
            tc.tile_pool(name="psum", bufs=pbufs, space="PSUM") as ppool,
        ):
            idt = cpool.tile([128, 128], f32)
            nc.sync.dma_start(idt[:], ident[:])
            # weight halves: w[k][h] = MkT[h*128:(h+1)*128, :]
            wt = []
            for k, mt in enumerate((m0t, m1t)):
                row = []
                for h in range(2):
                    w = cpool.tile([128, OUT_DIM], f32, tag=f"w{k}{h}")
                    nc.sync.dma_start(w[:], mt[h * 128:(h + 1) * 128, :])
                    row.append(w)
                wt.append(row)

            for c in range(chunks):
                xv = xs[c * chunk_tok:(c + 1) * chunk_tok, :].rearrange(
                    "(a p) i -> p a i", p=128
                )
                xt = pool.tile([128, blocks * IN_DIM], f32, tag="xt")
                nc.sync.dma_start(xt[:].rearrange("p (a i) -> p a i", i=IN_DIM), xv)

                # transpose x blocks: XT[h] = x[:, h-half].T, packed [128i, blocks*128tok]
                xtp0 = ppool.tile([128, blocks * 128], f32, tag="xtp0")
                xtp1 = ppool.tile([128, blocks * 128], f32, tag="xtp1")
                xtp = [xtp0, xtp1]
                for a in range(blocks):
                    for h in range(2):
                        nc.tensor.transpose(
                            xtp[h][:, a * 128:(a + 1) * 128],
                            xt[:, a * IN_DIM + h * 128:a * IN_DIM + (h + 1) * 128],
                            idt[:],
                        )
                # tanh fused with PSUM->SBUF: t2 cols = (h, tok)
                t2 = pool.tile([128, 2 * blocks * 128], f32, tag="t2")
                half = blocks * 128
                for h in range(2):
                    nc.scalar.activation(t2[:, h * half:(h + 1) * half],
                                         xtp[h][:], Tanh)

                es = []
                for k, b in enumerate((1.5, 0.5)):
                    r3 = pool.tile([128, 2 * half], f32, tag=f"r3_{k}")
                    nc.vector._custom_dve(
                        KAN_R3, out=r3[:], in0=t2[:],
                        s0=3.5, s1=2.0 - b, imm2=2.0 + b,
                    )
                    ek = pool.tile([128, 2 * half], f32, tag=f"e_{k}")
                    nc.vector._custom_dve(
                        KAN_S3A, out=ek[:], in0=t2[:], in1=r3[:],
                        s0=3.5 * CP, s1=CP * (1.0 - b), imm2=CP * (1.0 + b),
                    )
                    es.append(ek)

                ov = out[c * chunk_tok:(c + 1) * chunk_tok, :].rearrange(
                    "(a p) o -> p a o", p=128
                )
                os_t = pool.tile([128, blocks * OUT_DIM], f32, tag="os")
                for a in range(blocks):
                    acc = ppool.tile([128, OUT_DIM], f32, tag="acc")
                    for j in range(4):
                        k, h = j // 2, j % 2
                        nc.tensor.matmul(
                            acc[:],
                            es[k][:, h * half + a * 128:h * half + (a + 1) * 128],
                            wt[k][h][:],
                            start=(j == 0),
                            stop=(j == 3),
                        )
                    nc.scalar.copy(os_t[:, a * OUT_DIM:(a + 1) * OUT_DIM], acc[:])
                nc.sync.dma_start(ov, os_t[:].rearrange("p (a o) -> p a o", o=OUT_DIM))

    nc.compile()
    return nc


def _get_nc():
    if "nc" not in _CACHE:
        _CACHE["nc"] = _build_bass(SBUFS, PBUFS, CHUNK_TOK_RT)
    return _CACHE["nc"]


SBUFS = 6
PBUFS = 2
CHUNK_TOK_RT = 256


def kernel(x, inner_coeffs, outer_coeffs):
    from concourse import bass_utils

    x = np.asarray(x, dtype=np.float32)
    inner = np.asarray(inner_coeffs, dtype=np.float32)
    outer = np.asarray(outer_coeffs, dtype=np.float32)

    B, S, I = x.shape
    xf = np.ascontiguousarray(x.reshape(B * S, I))

    # MkT[i,o] = W[o,i] * c_k[i] / 6
    m0 = np.ascontiguousarray((outer.T * inner[:, 0:1]) / 6.0).astype(np.float32)
    m1 = np.ascontiguousarray((outer.T * inner[:, 1:2]) / 6.0).astype(np.float32)
    ident = np.eye(128, dtype=np.float32)

    nc = _get_nc()
    in_maps = []
    for i in range(N_CORES):
        in_maps.append({
            "xs": xf[i * TOK_PER_CORE:(i + 1) * TOK_PER_CORE],
            "m0t": m0, "m1t": m1, "ident": ident,
        })
    res = bass_utils.run_bass_kernel_spmd(nc, in_maps, list(range(N_CORES)))
    outs = [res.results[i]["out"] for i in range(N_CORES)]
    full = np.concatenate(outs, axis=0).reshape(B, S, OUT_DIM)
    return full
